# revision 2
# baseline (speedup 1.0000x reference)
"""Bass/Tile kernel for a 3-layer bidirectional LSTM classifier on 8 TRN2 cores.

Problem shapes (hardcoded): x [256, 512, 16], H=256, 3 BiLSTM layers, fc -> [256].

Strategy: data-parallel over batch (B=32 per core, no collectives). Per core,
the fwd and rev recurrences of each layer run as two independent interleaved
streams. All state is kept transposed (hT [H, B], gates [4H, B]) so the
recurrence matmul keeps the recurrent weights as the PE-stationary operand and
no transposes are ever needed. Gate rows are pre-permuted host-side to
[i, f, o, g] chunk order so a single sigmoid covers chunks 0..5 and a single
tanh covers chunks 6..7.

Input projections (the large, parallel-over-T matmuls) are precomputed per
layer at N=512 in float32r (full PE rate) and staged in DRAM as bf16; the
sequential recurrence then only does the 16 small [128,128]x[128,32] fp32
matmuls + elementwise per step.
"""

import os
from contextlib import ExitStack

import numpy as np

import concourse.bass as bass
import concourse.mybir as mybir
import concourse.tile as tile
from concourse import bacc, bass_utils
from concourse.bass import ds

f32 = mybir.dt.float32
f32r = mybir.dt.float32r
bf16 = mybir.dt.bfloat16
AF = mybir.ActivationFunctionType

H = 256
G = 1024  # 4H
NCORES = 8
BFULL = 256
TFULL = 512
I0 = 16

# gate chunk order i,i,f,f,o,o,g,g (PyTorch order in weights is i,f,g,o)
_PERM = np.concatenate(
    [np.arange(0, 512), np.arange(768, 1024), np.arange(512, 768)]
)


def _prep_wih(w):
    """[1024, Din] -> stationary layout [min(Din,128), nk*1024], chunk (k, m)
    at cols k*1024 + m*128; lhsT[kk, m*128+mm] = w_perm[m*128+mm, k*128+kk]."""
    wr = np.asarray(w, np.float32)[_PERM]
    din = wr.shape[1]
    if din <= 128:
        return np.ascontiguousarray(wr.T)
    nk = din // 128
    out = np.empty((128, nk * 1024), np.float32)
    for k in range(nk):
        out[:, k * 1024 : (k + 1) * 1024] = wr[:, k * 128 : (k + 1) * 128].T
    return out


def _prep_b(b):
    return np.ascontiguousarray(np.asarray(b, np.float32)[_PERM].reshape(8, 128).T)


def build(nc, T=TFULL, B=32):
    """Emit the full per-core program into nc (a Bacc)."""
    TB = T * B
    U = 16  # rec unroll / proj block (timesteps)
    NBLK = T // U
    UB = U * B

    xT0 = nc.dram_tensor("xT0", [I0, TB], f32, kind="ExternalInput").ap()
    win = {}
    for l in range(3):
        kp, kch = (I0, 1) if l == 0 else (128, 4)
        for d, dn in enumerate("fr"):
            win[(l, d, "wih")] = nc.dram_tensor(
                f"wih{l}{dn}", [kp, kch * 1024], f32, kind="ExternalInput"
            ).ap()
            win[(l, d, "whh")] = nc.dram_tensor(
                f"whh{l}{dn}", [128, 2048], f32, kind="ExternalInput"
            ).ap()
            win[(l, d, "b")] = nc.dram_tensor(
                f"b{l}{dn}", [128, 8], f32, kind="ExternalInput"
            ).ap()
    out_h2f = nc.dram_tensor("h2f", [128, 2 * B], f32, kind="ExternalOutput").ap()
    out_h2r = nc.dram_tensor("h2r", [128, 2 * B], f32, kind="ExternalOutput").ap()

    with tile.TileContext(nc) as tc, ExitStack() as ctx:
        dram = ctx.enter_context(tc.tile_pool(name="dram", bufs=1, space="DRAM"))
        wpool = ctx.enter_context(tc.tile_pool(name="wts", bufs=1))
        mvpool = ctx.enter_context(tc.tile_pool(name="mv", bufs=3))
        pj_psum = ctx.enter_context(tc.tile_pool(name="pjps", bufs=2, space="PSUM"))
        xpspool = ctx.enter_context(tc.tile_pool(name="xps", bufs=3))
        xppool = ctx.enter_context(tc.tile_pool(name="xpt", bufs=4))
        rps = [
            ctx.enter_context(tc.tile_pool(name=f"rps{d}", bufs=2, space="PSUM"))
            for d in range(2)
        ]
        gpool = ctx.enter_context(tc.tile_pool(name="g", bufs=3))
        state = ctx.enter_context(tc.tile_pool(name="st", bufs=1))
        tmp = ctx.enter_context(tc.tile_pool(name="tmp", bufs=3))

        # DRAM scratch
        xp = {}
        for l in range(3):
            for d in range(2):
                nt = U if (l == 2 and d == 1) else T
                xp[(l, d)] = dram.tile([128, nt, 8 * B], bf16, tag=f"xp{l}{d}", name=f"xp{l}{d}")
        xin = {
            1: dram.tile([4, 128, TB], f32, tag="xin1", name="xin1"),
            2: dram.tile([4, 128, TB], f32, tag="xin2", name="xin2"),
        }

        def load_weights(l):
            kp, kch = (I0, 1) if l == 0 else (128, 4)
            wt = {}
            for d in range(2):
                wih_t = wpool.tile([kp, kch * 1024], f32r, tag=f"wih{d}")
                nc.sync.dma_start(wih_t[:], win[(l, d, "wih")][:].bitcast(f32r))
                whh_t = wpool.tile([128, 2048], f32, tag=f"whh{d}")
                nc.sync.dma_start(whh_t[:], win[(l, d, "whh")][:])
                b_t = wpool.tile([128, 8], f32, tag=f"b{d}")
                nc.sync.dma_start(b_t[:], win[(l, d, "b")][:])
                wt[d] = (wih_t, whh_t, b_t)
            return wt

        def emit_proj_block(l, wt, jb, dirs):
            """One real-time block jb (16 timesteps, UB cols) of the input
            projection for layer l, for the given target dirs."""
            kp, kch = (I0, 1) if l == 0 else (128, 4)
            mvs = []
            for k in range(kch):
                mv = mvpool.tile([kp, UB], f32r, tag=f"mv{k}")
                if l == 0:
                    nc.sync.dma_start(mv[:], xT0[:, ds(jb * UB, UB)].bitcast(f32r))
                else:
                    col = ds(jb * UB, UB) if k < 2 else ds(
                        (NBLK - 1) * UB - jb * UB, UB
                    )
                    nc.sync.dma_start(mv[:], xin[l][k, :, col].bitcast(f32r))
                mvs.append(mv)
            for d in dirs:
                wih_t, _, b_t = wt[d]
                for m in range(8):
                    ps = pj_psum.tile([128, UB], f32)
                    for k in range(kch):
                        straight = (d == 0) if (l == 0 or k < 2) else (d == 1)
                        rhs = (
                            mvs[k][:]
                            if straight
                            else mvs[k][:]
                            .rearrange("p (t b) -> p t b", b=B)[:, ::-1, :]
                        )
                        nc.tensor.matmul(
                            ps[:],
                            wih_t[:, (k * 8 + m) * 128 : (k * 8 + m + 1) * 128],
                            rhs,
                            start=(k == 0),
                            stop=(k == kch - 1),
                        )
                    xps = xpspool.tile([128, UB], bf16)
                    nc.scalar.activation(
                        xps[:], ps[:], AF.Identity, bias=b_t[:, m : m + 1]
                    )
                    dst_row = ds(jb * U, U) if d == 0 else ds(
                        (NBLK - 1) * U - jb * U, U
                    )
                    dst = xp[(l, d)][:, dst_row, m * B : (m + 1) * B]
                    src = xps[:].rearrange("p (t b) -> p t b", b=B)
                    nc.sync.dma_start(dst, src)

        def proj_layer(l, wt, dirs=(0, 1)):
            with tc.For_i(0, NBLK, 1) as jb:
                emit_proj_block(l, wt, jb, dirs)

        def cell_step(l, d, wt, jexpr, colexpr, h, c, store):
            _, whh_t, _ = wt[d]
            xpt = xppool.tile([128, 8 * B], bf16, tag=f"xp{d}")
            nc.sync.dma_start(xpt[:], xp[(l, d)][:, ds(jexpr, 1), :])
            ps = rps[d].tile([128, 8 * B], f32)
            for m in range(8):
                for k in range(2):
                    nc.tensor.matmul(
                        ps[:, m * B : (m + 1) * B],
                        whh_t[:, (k * 8 + m) * 128 : (k * 8 + m + 1) * 128],
                        h[:, k * B : (k + 1) * B],
                        start=(k == 0),
                        stop=(k == 1),
                    )
            g = gpool.tile([128, 8 * B], f32, tag=f"g{d}")
            nc.vector.tensor_add(g[:], ps[:], xpt[:])
            sg = gpool.tile([128, 6 * B], f32, tag=f"sg{d}")
            nc.scalar.activation(sg[:], g[:, 0 : 6 * B], AF.Sigmoid)
            tg = gpool.tile([128, 2 * B], f32, tag=f"tg{d}")
            nc.scalar.activation(tg[:], g[:, 6 * B : 8 * B], AF.Tanh)
            ta = tmp.tile([128, 2 * B], f32, tag=f"ta{d}")
            nc.gpsimd.tensor_mul(ta[:], sg[:, 2 * B : 4 * B], c[:])  # f*c
            tb = tmp.tile([128, 2 * B], f32, tag=f"tb{d}")
            nc.vector.tensor_mul(tb[:], sg[:, 0 : 2 * B], tg[:])  # i*g
            nc.vector.tensor_add(c[:], ta[:], tb[:])
            tcb = tmp.tile([128, 2 * B], f32, tag=f"tc{d}")
            nc.scalar.activation(tcb[:], c[:], AF.Tanh)
            nc.gpsimd.tensor_mul(h[:], sg[:, 4 * B : 6 * B], tcb[:])  # o*tanh(c)
            if store:
                for jh in range(2):
                    nc.sync.dma_start(
                        xin[l + 1][2 * d + jh, :, ds(colexpr, B)],
                        h[:, jh * B : (jh + 1) * B],
                    )

        def rec_layer(l, wt, dirs=(0, 1), store=True):
            hs, cs = {}, {}
            for d in dirs:
                hh = state.tile([128, 2 * B], f32, tag=f"h{d}")
                cc = state.tile([128, 2 * B], f32, tag=f"c{d}")
                nc.gpsimd.memset(hh[:], 0.0)
                nc.gpsimd.memset(cc[:], 0.0)
                hs[d], cs[d] = hh, cc
            with tc.For_i(
                0, T, U, hint_engines=(mybir.EngineType.PE,)
            ) as j0:
                jcol = j0 * B
                for s in range(U):
                    for d in dirs:
                        cell_step(
                            l, d, wt, j0 + s, jcol + s * B, hs[d], cs[d], store
                        )
            return hs, cs

        # ---- layer 0 ----
        wt = load_weights(0)
        proj_layer(0, wt)
        rec_layer(0, wt)
        # ---- layer 1 ----
        wt = load_weights(1)
        proj_layer(1, wt)
        rec_layer(1, wt)
        # ---- layer 2 ----
        wt = load_weights(2)
        proj_layer(2, wt, dirs=(0,))
        emit_proj_block(2, wt, NBLK - 1, dirs=(1,))
        hs, _ = rec_layer(2, wt, dirs=(0,), store=False)
        nc.sync.dma_start(out_h2f[:], hs[0][:])
        # layer-2 reverse: only its first step (t = T-1) feeds the output
        hr = state.tile([128, 2 * B], f32, tag="h1")
        cr = state.tile([128, 2 * B], f32, tag="c1")
        nc.gpsimd.memset(hr[:], 0.0)
        nc.gpsimd.memset(cr[:], 0.0)
        cell_step(2, 1, wt, 0, 0, hr, cr, False)
        nc.sync.dma_start(out_h2r[:], hr[:])


def _make_in_maps(inputs, T=TFULL, B=32, ncores=NCORES):
    x = np.ascontiguousarray(np.asarray(inputs["x"], np.float32))
    shared = {}
    for l in range(3):
        for d, dn in enumerate("fr"):
            shared[f"wih{l}{dn}"] = _prep_wih(inputs[f"wih{l}{dn}"])
            shared[f"whh{l}{dn}"] = _prep_wih(inputs[f"whh{l}{dn}"])
            shared[f"b{l}{dn}"] = _prep_b(inputs[f"b{l}{dn}"])
    in_maps = []
    for ci in range(ncores):
        xs = x[ci * B : (ci + 1) * B, :T]  # [B, T, 16]
        xt = np.ascontiguousarray(xs.transpose(2, 1, 0).reshape(I0, T * B))
        m = dict(shared)
        m["xT0"] = xt
        in_maps.append(m)
    return in_maps


def _assemble(results, inputs, B=32):
    fcw = np.asarray(inputs["fcw"], np.float32)[0]
    fcb = float(np.asarray(inputs["fcb"], np.float32)[0])
    out = np.empty(len(results) * B, np.float32)
    for ci, r in enumerate(results):
        h2f = np.concatenate([r["h2f"][:, :B], r["h2f"][:, B:]], axis=0)
        h2r = np.concatenate([r["h2r"][:, :B], r["h2r"][:, B:]], axis=0)
        out[ci * B : (ci + 1) * B] = fcw[:256] @ h2f + fcw[256:] @ h2r + fcb
    return out


def kernel(**inputs):
    nc = bacc.Bacc(
        "TRN2", target_bir_lowering=False, debug=False, num_devices=NCORES
    )
    build(nc)
    nc.compile()
    in_maps = _make_in_maps(inputs)
    trace = os.environ.get("KERNEL_TRACE", "0") == "1"
    res = bass_utils.run_bass_kernel_spmd(
        nc, in_maps, core_ids=list(range(NCORES)), trace=trace
    )
    if trace and res.exec_time_ns is not None:
        print(f"HW exec time: {res.exec_time_ns} ns")
        if res.instructions_and_trace is not None:
            print(f"trace path: {res.instructions_and_trace[1]}")
    return _assemble(res.results, inputs)



# revision 12
# speedup vs baseline: 2.0668x; 2.0668x over previous
"""Bass/Tile kernel for a 3-layer bidirectional LSTM classifier on 8 TRN2 cores.

Problem shapes (hardcoded): x [256, 512, 16], H=256, 3 BiLSTM layers, fc -> [256].

Strategy: data-parallel over batch (B=32 per core, no collectives). Per core,
the fwd and rev recurrences of each layer run as two independent interleaved
streams. All state is kept transposed (hT [H, B], gates [4H, B]) so the
recurrence matmul keeps the recurrent weights as the PE-stationary operand and
no transposes are ever needed. Gate rows are pre-permuted host-side to
[i, f, o, g] chunk order so a single sigmoid covers chunks 0..5 and a single
tanh covers chunks 6..7.

Input projections (the large, parallel-over-T matmuls) are precomputed per
layer at N=512 in float32r (full PE rate) and staged in DRAM as bf16; the
sequential recurrence then only does the 16 small [128,128]x[128,32] fp32
matmuls + elementwise per step.
"""

import os
from contextlib import ExitStack

import ml_dtypes
import numpy as np

import concourse.bass as bass
import concourse.mybir as mybir
import concourse.tile as tile
from concourse import bacc, bass_utils
from concourse.bass import ds

f32 = mybir.dt.float32
f32r = mybir.dt.float32r
bf16 = mybir.dt.bfloat16
AF = mybir.ActivationFunctionType

H = 256
G = 1024  # 4H
NCORES = 8
BFULL = 256
TFULL = 512
I0 = 16

# gate chunk order i,i,f,f,o,o,g,g (PyTorch order in weights is i,f,g,o)
_PERM = np.concatenate(
    [np.arange(0, 512), np.arange(768, 1024), np.arange(512, 768)]
)


def _prep_wih(w):
    """[1024, Din] -> stationary layout [min(Din,128), nk*1024], chunk (k, m)
    at cols k*1024 + m*128; lhsT[kk, m*128+mm] = w_perm[m*128+mm, k*128+kk]."""
    wr = np.asarray(w, np.float32)[_PERM]
    din = wr.shape[1]
    if din <= 128:
        return np.ascontiguousarray(wr.T).astype(ml_dtypes.bfloat16)
    nk = din // 128
    out = np.empty((128, nk * 1024), np.float32)
    for k in range(nk):
        out[:, k * 1024 : (k + 1) * 1024] = wr[:, k * 128 : (k + 1) * 128].T
    return out.astype(ml_dtypes.bfloat16)


def _prep_b(b):
    return np.ascontiguousarray(np.asarray(b, np.float32)[_PERM].reshape(8, 128).T)


def build(nc, T=TFULL, B=32):
    """Emit the full per-core program into nc (a Bacc)."""
    TB = T * B
    U = 16  # rec unroll / proj block (timesteps)
    NBLK = T // U
    UB = U * B

    xT0 = nc.dram_tensor("xT0", [I0, TB], bf16, kind="ExternalInput").ap()
    win = {}
    for l in range(3):
        kp, kch = (I0, 1) if l == 0 else (128, 4)
        for d, dn in enumerate("fr"):
            win[(l, d, "wih")] = nc.dram_tensor(
                f"wih{l}{dn}", [kp, kch * 1024], bf16, kind="ExternalInput"
            ).ap()
            win[(l, d, "whh")] = nc.dram_tensor(
                f"whh{l}{dn}", [128, 2048], bf16, kind="ExternalInput"
            ).ap()
            win[(l, d, "b")] = nc.dram_tensor(
                f"b{l}{dn}", [128, 8], f32, kind="ExternalInput"
            ).ap()
    out_h2f = nc.dram_tensor("h2f", [128, 2 * B], bf16, kind="ExternalOutput").ap()
    out_h2r = nc.dram_tensor("h2r", [128, 2 * B], bf16, kind="ExternalOutput").ap()

    with tile.TileContext(nc) as tc, ExitStack() as ctx:
        dram = ctx.enter_context(tc.tile_pool(name="dram", bufs=1, space="DRAM"))
        wpool = ctx.enter_context(tc.tile_pool(name="wts", bufs=1))
        mvpool = ctx.enter_context(tc.tile_pool(name="mv", bufs=3))
        pj_psum = ctx.enter_context(tc.tile_pool(name="pjps", bufs=2, space="PSUM"))
        xpspool = ctx.enter_context(tc.tile_pool(name="xps", bufs=3))
        xppool = ctx.enter_context(tc.tile_pool(name="xpt", bufs=4))
        rps = [
            ctx.enter_context(tc.tile_pool(name=f"rps{d}", bufs=2, space="PSUM"))
            for d in range(2)
        ]
        gpool = ctx.enter_context(tc.tile_pool(name="g", bufs=3))
        state = ctx.enter_context(tc.tile_pool(name="st", bufs=1))
        tmp = ctx.enter_context(tc.tile_pool(name="tmp", bufs=3))

        # DRAM scratch
        xp = {}
        for l in range(3):
            for d in range(2):
                nt = U if (l == 2 and d == 1) else T
                xp[(l, d)] = dram.tile([128, nt, 8 * B], bf16, tag=f"xp{l}{d}", name=f"xp{l}{d}")
        xin = {
            1: dram.tile([4, 128, TB], bf16, tag="xin1", name="xin1"),
            2: dram.tile([4, 128, TB], bf16, tag="xin2", name="xin2"),
        }

        def load_weights(l):
            kp, kch = (I0, 1) if l == 0 else (128, 4)
            wt = {}
            for d in range(2):
                wih_t = wpool.tile([kp, kch * 1024], bf16, tag=f"wih{d}")
                nc.sync.dma_start(wih_t[:], win[(l, d, "wih")][:])
                whh_t = wpool.tile([128, 2048], bf16, tag=f"whh{d}")
                nc.sync.dma_start(whh_t[:], win[(l, d, "whh")][:])
                b_t = wpool.tile([128, 8], f32, tag=f"b{d}")
                nc.sync.dma_start(b_t[:], win[(l, d, "b")][:])
                wt[d] = (wih_t, whh_t, b_t)
            return wt

        def emit_proj_block(l, wt, jb, dirs):
            """One real-time block jb (16 timesteps, UB cols) of the input
            projection for layer l, for the given target dirs."""
            kp, kch = (I0, 1) if l == 0 else (128, 4)
            mvs = []
            for k in range(kch):
                mv = mvpool.tile([kp, UB], bf16, tag=f"mv{k}")
                if l == 0:
                    nc.sync.dma_start(mv[:], xT0[:, ds(jb * UB, UB)])
                else:
                    col = ds(jb * UB, UB) if k < 2 else ds(
                        (NBLK - 1) * UB - jb * UB, UB
                    )
                    nc.sync.dma_start(mv[:], xin[l][k, :, col])
                mvs.append(mv)
            for d in dirs:
                wih_t, _, b_t = wt[d]
                for m in range(8):
                    ps = pj_psum.tile([128, UB], f32)
                    for k in range(kch):
                        straight = (d == 0) if (l == 0 or k < 2) else (d == 1)
                        rhs = (
                            mvs[k][:]
                            if straight
                            else mvs[k][:]
                            .rearrange("p (t b) -> p t b", b=B)[:, ::-1, :]
                        )
                        nc.tensor.matmul(
                            ps[:],
                            wih_t[:, (k * 8 + m) * 128 : (k * 8 + m + 1) * 128],
                            rhs,
                            start=(k == 0),
                            stop=(k == kch - 1),
                        )
                    xps = xpspool.tile([128, UB], bf16)
                    nc.scalar.activation(
                        xps[:], ps[:], AF.Identity, bias=b_t[:, m : m + 1]
                    )
                    dst_row = ds(jb * U, U) if d == 0 else ds(
                        (NBLK - 1) * U - jb * U, U
                    )
                    dst = xp[(l, d)][:, dst_row, m * B : (m + 1) * B]
                    src = xps[:].rearrange("p (t b) -> p t b", b=B)
                    nc.sync.dma_start(dst, src)

        def proj_layer(l, wt, dirs=(0, 1)):
            with tc.For_i(0, NBLK, 1) as jb:
                emit_proj_block(l, wt, jb, dirs)

        def cell_step(l, d, wt, jexpr, colexpr, h, c, store):
            _, whh_t, _ = wt[d]
            xpt = xppool.tile([128, 8 * B], bf16, tag=f"xp{d}")
            nc.sync.dma_start(xpt[:], xp[(l, d)][:, ds(jexpr, 1), :])
            ps = rps[d].tile([128, 8 * B], f32)
            for m in range(8):
                for k in range(2):
                    nc.tensor.matmul(
                        ps[:, m * B : (m + 1) * B],
                        whh_t[:, (k * 8 + m) * 128 : (k * 8 + m + 1) * 128],
                        h[:, k * B : (k + 1) * B],
                        start=(k == 0),
                        stop=(k == 1),
                    )
            g = gpool.tile([128, 8 * B], f32, tag=f"g{d}")
            nc.vector.tensor_add(g[:], ps[:], xpt[:])
            sg = gpool.tile([128, 6 * B], f32, tag=f"sg{d}")
            nc.scalar.activation(sg[:], g[:, 0 : 6 * B], AF.Sigmoid)
            tg = gpool.tile([128, 2 * B], f32, tag=f"tg{d}")
            nc.scalar.activation(tg[:], g[:, 6 * B : 8 * B], AF.Tanh)
            ta = tmp.tile([128, 2 * B], f32, tag=f"ta{d}")
            nc.gpsimd.tensor_mul(ta[:], sg[:, 2 * B : 4 * B], c[:])  # f*c
            tb = tmp.tile([128, 2 * B], f32, tag=f"tb{d}")
            nc.vector.tensor_mul(tb[:], sg[:, 0 : 2 * B], tg[:])  # i*g
            nc.vector.tensor_add(c[:], ta[:], tb[:])
            tcb = tmp.tile([128, 2 * B], f32, tag=f"tc{d}")
            nc.scalar.activation(tcb[:], c[:], AF.Tanh)
            nc.gpsimd.tensor_mul(h[:], sg[:, 4 * B : 6 * B], tcb[:])  # o*tanh(c)
            if store:
                for jh in range(2):
                    nc.sync.dma_start(
                        xin[l + 1][2 * d + jh, :, ds(colexpr, B)],
                        h[:, jh * B : (jh + 1) * B],
                    )

        def rec_layer(l, wt, dirs=(0, 1), store=True):
            hs, cs = {}, {}
            for d in dirs:
                hh = state.tile([128, 2 * B], bf16, tag=f"h{d}")
                cc = state.tile([128, 2 * B], f32, tag=f"c{d}")
                nc.gpsimd.memset(hh[:], 0.0)
                nc.gpsimd.memset(cc[:], 0.0)
                hs[d], cs[d] = hh, cc
            with tc.For_i(
                0, T, U, hint_engines=(mybir.EngineType.PE,)
            ) as j0:
                jcol = j0 * B
                for s in range(U):
                    for d in dirs:
                        cell_step(
                            l, d, wt, j0 + s, jcol + s * B, hs[d], cs[d], store
                        )
            return hs, cs

        # ---- layer 0 ----
        wt = load_weights(0)
        proj_layer(0, wt)
        rec_layer(0, wt)
        # ---- layer 1 ----
        wt = load_weights(1)
        proj_layer(1, wt)
        rec_layer(1, wt)
        # ---- layer 2 ----
        wt = load_weights(2)
        proj_layer(2, wt, dirs=(0,))
        emit_proj_block(2, wt, NBLK - 1, dirs=(1,))
        hs, _ = rec_layer(2, wt, dirs=(0,), store=False)
        nc.sync.dma_start(out_h2f[:], hs[0][:])
        # layer-2 reverse: only its first step (t = T-1) feeds the output
        hr = state.tile([128, 2 * B], bf16, tag="h1")
        cr = state.tile([128, 2 * B], f32, tag="c1")
        nc.gpsimd.memset(hr[:], 0.0)
        nc.gpsimd.memset(cr[:], 0.0)
        cell_step(2, 1, wt, 0, 0, hr, cr, False)
        nc.sync.dma_start(out_h2r[:], hr[:])


def _make_in_maps(inputs, T=TFULL, B=32, ncores=NCORES):
    x = np.ascontiguousarray(np.asarray(inputs["x"], np.float32))
    shared = {}
    for l in range(3):
        for d, dn in enumerate("fr"):
            shared[f"wih{l}{dn}"] = _prep_wih(inputs[f"wih{l}{dn}"])
            shared[f"whh{l}{dn}"] = _prep_wih(inputs[f"whh{l}{dn}"])
            shared[f"b{l}{dn}"] = _prep_b(inputs[f"b{l}{dn}"])
    in_maps = []
    for ci in range(ncores):
        xs = x[ci * B : (ci + 1) * B, :T]  # [B, T, 16]
        xt = np.ascontiguousarray(xs.transpose(2, 1, 0).reshape(I0, T * B))
        m = dict(shared)
        m["xT0"] = xt.astype(ml_dtypes.bfloat16)
        in_maps.append(m)
    return in_maps


def _assemble(results, inputs, B=32):
    fcw = np.asarray(inputs["fcw"], np.float32)[0]
    fcb = float(np.asarray(inputs["fcb"], np.float32)[0])
    out = np.empty(len(results) * B, np.float32)
    for ci, r in enumerate(results):
        h2fr = np.asarray(r["h2f"], np.float32)
        h2rr = np.asarray(r["h2r"], np.float32)
        h2f = np.concatenate([h2fr[:, :B], h2fr[:, B:]], axis=0)
        h2r = np.concatenate([h2rr[:, :B], h2rr[:, B:]], axis=0)
        out[ci * B : (ci + 1) * B] = fcw[:256] @ h2f + fcw[256:] @ h2r + fcb
    return out


def kernel(**inputs):
    nc = bacc.Bacc(
        "TRN2", target_bir_lowering=False, debug=False, num_devices=NCORES
    )
    build(nc)
    nc.compile()
    in_maps = _make_in_maps(inputs)
    trace = os.environ.get("KERNEL_TRACE", "0") == "1"
    res = bass_utils.run_bass_kernel_spmd(
        nc, in_maps, core_ids=list(range(NCORES)), trace=trace
    )
    if trace and res.exec_time_ns is not None:
        print(f"HW exec time: {res.exec_time_ns} ns")
        if res.instructions_and_trace is not None:
            print(f"trace path: {res.instructions_and_trace[1]}")
    return _assemble(res.results, inputs)



# revision 24
# speedup vs baseline: 2.6162x; 1.2658x over previous
"""Bass/Tile kernel for a 3-layer bidirectional LSTM classifier on 8 TRN2 cores.

Problem shapes (hardcoded): x [256, 512, 16], H=256, 3 BiLSTM layers, fc -> [256].

Strategy: data-parallel over batch (B=32 per core, no collectives). Per layer,
the input projection is computed just-in-time in U=4-step blocks and written
directly into the same PSUM tiles the recurrence accumulates into (bias added
via tiny [1,128]x[1,UB] matmuls), so gates need no separate add/copy. Each
phase runs two independent recurrent streams (fwd+rev, or two batch halves for
the fwd-only last layer) with per-stream tiles so the dependency chains
overlap; per-step ops are emitted stage-interleaved across streams.

All PE operands are bf16. tanh(g) is folded into the one sigmoid over all 8
gate chunks by pre-scaling g-gate weights by 2 host-side (tanh(x) =
2*sigmoid(2x)-1, applied in the c-update arithmetic on DVE).

Layer-0 output history lives in SBUF (hbufF/hbufR) serving as both recurrent
state and layer-1's projection input; layer-1 output goes to DRAM (xin2) for
layer 2. Only the last timestep of layer 2 (fwd: last step of the forward
recurrence; rev: first step of the reverse recurrence) feeds the host-side fc.
"""

import os
from contextlib import ExitStack

import ml_dtypes
import numpy as np

import concourse.bass as bass
import concourse.mybir as mybir
import concourse.tile as tile
from concourse import bacc, bass_utils
from concourse.bass import ds

f32 = mybir.dt.float32
bf16 = mybir.dt.bfloat16
AF = mybir.ActivationFunctionType
ALU = mybir.AluOpType

H = 256
NCORES = 8
BFULL = 256
TFULL = 512
I0 = 16
U = 4  # timesteps per block

# gate chunk order i,i,f,f,o,o,g,g (PyTorch order in weights is i,f,g,o)
_PERM = np.concatenate(
    [np.arange(0, 512), np.arange(768, 1024), np.arange(512, 768)]
)


def _prep_wih(w):
    """[1024, Din] -> stationary layout [min(Din,128), nk*1024], chunk (k, m)
    at cols k*1024 + m*128; lhsT[kk, m*128+mm] = w_perm[m*128+mm, k*128+kk].
    g-gate rows (post-perm 768:1024) are pre-scaled by 2 for the
    tanh-via-sigmoid trick."""
    wr = np.asarray(w, np.float32)[_PERM].copy()
    wr[768:] *= 2.0
    din = wr.shape[1]
    if din <= 128:
        return np.ascontiguousarray(wr.T).astype(ml_dtypes.bfloat16)
    nk = din // 128
    out = np.empty((128, nk * 1024), np.float32)
    for k in range(nk):
        out[:, k * 1024 : (k + 1) * 1024] = wr[:, k * 128 : (k + 1) * 128].T
    return out.astype(ml_dtypes.bfloat16)


def _prep_b(b, UB=U * 32):
    """-> [128, 8*UB]: bias broadcast over a block's columns, per gate chunk."""
    br = np.asarray(b, np.float32)[_PERM].copy()
    br[768:] *= 2.0
    bm = br.reshape(8, 128).T  # [p, m]
    out = np.repeat(bm[:, :, None], UB, axis=2).reshape(128, 8 * UB)
    return np.ascontiguousarray(out).astype(ml_dtypes.bfloat16)


def build(nc, T=TFULL, B=32, debug_taps=False):
    NBLK = T // U
    UB = U * B
    TB = T * B
    PE = mybir.EngineType.PE

    dbg = {}
    if debug_taps:
        dbg["F"] = nc.dram_tensor("dbgF", [128, 2, TB], bf16, kind="ExternalOutput").ap()
        dbg["R"] = nc.dram_tensor("dbgR", [128, 2, TB], bf16, kind="ExternalOutput").ap()
        dbg["X2"] = nc.dram_tensor("dbgX2", [4, 128, TB], bf16, kind="ExternalOutput").ap()

    xT0 = nc.dram_tensor("xT0", [I0, TB], bf16, kind="ExternalInput").ap()
    win = {}
    for l in range(3):
        kp, kch = (I0, 1) if l == 0 else (128, 4)
        for d, dn in enumerate("fr"):
            win[(l, d, "wih")] = nc.dram_tensor(
                f"wih{l}{dn}", [kp, kch * 1024], bf16, kind="ExternalInput"
            ).ap()
            win[(l, d, "whh")] = nc.dram_tensor(
                f"whh{l}{dn}", [128, 2048], bf16, kind="ExternalInput"
            ).ap()
            win[(l, d, "b")] = nc.dram_tensor(
                f"b{l}{dn}", [128, 8 * U * 32], bf16, kind="ExternalInput"
            ).ap()
    out_h2f = nc.dram_tensor("h2f", [128, 2 * B], bf16, kind="ExternalOutput").ap()
    out_h2r = nc.dram_tensor("h2r", [128, 2 * B], bf16, kind="ExternalOutput").ap()

    with tile.TileContext(nc) as tc, ExitStack() as ctx:
        dram = ctx.enter_context(tc.tile_pool(name="dram", bufs=1, space="DRAM"))
        hpool = ctx.enter_context(tc.tile_pool(name="hist", bufs=1))
        wpool = ctx.enter_context(tc.tile_pool(name="wts", bufs=1))
        mvpool = ctx.enter_context(tc.tile_pool(name="mv", bufs=1))
        pspool = ctx.enter_context(tc.tile_pool(name="ps", bufs=1, space="PSUM"))
        spool = ctx.enter_context(tc.tile_pool(name="sp", bufs=1))
        stpool = ctx.enter_context(tc.tile_pool(name="st", bufs=1))

        xin2 = dram.tile([4, 128, TB], bf16, tag="xin2", name="xin2")

        zeros = stpool.tile([128, 2, B], bf16, tag="zeros", name="zeros")
        nc.gpsimd.memset(zeros[:], 0.0)
        ident_in = nc.dram_tensor("ident", [128, 128], bf16, kind="ExternalInput").ap()
        ident = stpool.tile([128, 128], bf16, tag="ident", name="ident")
        nc.sync.dma_start(ident[:], ident_in[:])
        # layer-0 output history: fwd chunks / rev chunks, by real time
        hbufF = hpool.tile([128, 2, TB], bf16, tag="hbufF", name="hbufF")
        hbufR = hpool.tile([128, 2, TB], bf16, tag="hbufR", name="hbufR")

        def load_w(l):
            kp, kch = (I0, 1) if l == 0 else (128, 4)
            wt = {}
            for d in range(2):
                wih_t = wpool.tile(
                    [kp, kch * 1024], bf16, tag=f"wih{d}", name=f"wih{d}"
                )
                nc.sync.dma_start(wih_t[:], win[(l, d, "wih")][:])
                whh_t = wpool.tile([128, 2048], bf16, tag=f"whh{d}", name=f"whh{d}")
                nc.sync.dma_start(whh_t[:], win[(l, d, "whh")][:])
                b_t = wpool.tile([128, 8 * UB], bf16, tag=f"b{d}", name=f"b{d}")
                nc.sync.dma_start(b_t[:], win[(l, d, "b")][:])
                wt[d] = (wih_t, whh_t, b_t)
            return wt

        def phase(l, streams, store_l1=False, final_dma=None):
            """One layer phase. streams: list of (d, g0, gw) — direction and
            batch-column slice [g0, g0+gw) each independent recurrent chain
            owns. Blocks of U steps; per-stream PSUM tiles rotate over 2
            parities; proj for block jb+1 is emitted interleaved with the
            recurrence of block jb."""
            kp, kch = (I0, 1) if l == 0 else (128, 4)
            wt = load_w(l)
            ns = len(streams)

            # per-stream tiles
            pt = {}  # (si, parity) -> PSUM gates tile [128, 8, U*gw]
            sg = {}
            ta = {}
            tb2 = {}
            tcb = {}
            ct = {}
            slots = {}
            for si, (d, g0, gw) in enumerate(streams):
                for p in range(2):
                    pt[(si, p)] = pspool.tile(
                        [128, 8, U * gw], f32, tag=f"pt{si}{p}", name=f"pt{si}{p}"
                    )
                    sg[(si, p)] = spool.tile(
                        [128, 8, gw], f32, tag=f"sg{si}{p}", name=f"sg{si}{p}"
                    )
                    ta[(si, p)] = spool.tile(
                        [128, 2, gw], f32, tag=f"ta{si}{p}", name=f"ta{si}{p}"
                    )
                    tb2[(si, p)] = spool.tile(
                        [128, 2, gw], f32, tag=f"tb{si}{p}", name=f"tb{si}{p}"
                    )
                    tcb[(si, p)] = spool.tile(
                        [128, 2, gw], f32, tag=f"tc{si}{p}", name=f"tc{si}{p}"
                    )
                ct[si] = stpool.tile([128, 2, gw], f32, tag=f"c{si}", name=f"c{si}")
                nc.vector.memset(ct[si][:], 0.0)
                if l > 0:
                    slots[si] = [
                        stpool.tile(
                            [128, 2, gw], bf16, tag=f"sl{si}{p}", name=f"sl{si}{p}"
                        )
                        for p in range(2)
                    ]

            # mv tiles (L0/L2 only): per (stream, k, parity)
            mvt = {}
            if l != 1:
                for si, (d, g0, gw) in enumerate(streams):
                    if l == 2 and si > 0:
                        break  # L2 groups share one direction's mv tiles
                    for k in range(kch):
                        for p in range(2):
                            mvt[(d, k, p)] = mvpool.tile(
                                [kp, UB], bf16, tag=f"mv{d}{k}{p}", name=f"mv{d}{k}{p}"
                            )

            def mv_dma(jb, parity):
                if l == 1:
                    return
                done = set()
                for si, (d, g0, gw) in enumerate(streams):
                    if d in done:
                        continue
                    done.add(d)
                    for k in range(kch):
                        col = (
                            ds(jb * UB, UB)
                            if d == 0
                            else ds((NBLK - 1) * UB - jb * UB, UB)
                        )
                        src = xT0[:, col] if l == 0 else xin2[k, :, col]
                        nc.sync.dma_start(mvt[(d, k, parity)][:], src)

            def proj_rhs(d, k, jb, parity, g0, gw):
                if l == 1:
                    col = (
                        ds(jb * UB, UB)
                        if d == 0
                        else ds((NBLK - 1) * UB - jb * UB, UB)
                    )
                    base = (hbufF if k < 2 else hbufR)[:, k % 2, col]
                else:
                    base = mvt[(d, k, parity)][:]
                r3 = base.rearrange("p (t b) -> p t b", b=B)
                if d == 1:
                    r3 = r3[:, ::-1, :]
                if gw != B:
                    r3 = r3[:, :, g0 : g0 + gw]
                return r3

            def proj_ops(jb, parity):
                ops = []
                for si, (d, g0, gw) in enumerate(streams):
                    p = pt[(si, parity)]
                    _, _, b_t = wt[d]
                    bv = b_t[:].rearrange("p (m c) -> p m c", c=UB)
                    for half in range(2):
                        ops.append(
                            (
                                lambda p=p, half=half, bv=bv, gw=gw: nc.tensor.matmul(
                                    p[:, 4 * half : 4 * half + 4, :],
                                    ident[:],
                                    bv[:, 4 * half : 4 * half + 4, 0 : U * gw],
                                    start=True,
                                    stop=False,
                                    skip_group_check=True,
                                )
                            )
                        )
                    for k in range(kch):
                        rhs = proj_rhs(d, k, jb, parity, g0, gw)
                        wih_t = wt[d][0]
                        for m in range(8):
                            ops.append(
                                (
                                    lambda p=p, m=m, k=k, rhs=rhs, wih_t=wih_t: nc.tensor.matmul(
                                        p[:, m, :],
                                        wih_t[:, (k * 8 + m) * 128 : (k * 8 + m + 1) * 128],
                                        rhs,
                                        start=False,
                                        stop=False,
                                        skip_group_check=True,
                                    )
                                )
                            )
                return ops

            def rec_rhs(si, d, g0, gw, k, jb, s):
                if isinstance(jb, int) and jb == 0 and s == 0:
                    return zeros[:, k, g0 : g0 + gw]
                if l == 0:
                    if d == 0:
                        col = jb * UB + (s - 1) * B + g0
                        return hbufF[:, k, ds(col, gw)]
                    col = T * B - jb * UB - s * B + g0
                    return hbufR[:, k, ds(col, gw)]
                return slots[si][(s - 1) % 2][:, k, :]

            def emit_block(jb, parity, next_ops):
                for s in range(U):
                    for si, (d, g0, gw) in enumerate(streams):
                        p = pt[(si, parity)]
                        whh_t = wt[d][1]
                        for m in range(8):
                            for k in range(2):
                                nc.tensor.matmul(
                                    p[:, m, ds(s * gw, gw)],
                                    whh_t[:, (k * 8 + m) * 128 : (k * 8 + m + 1) * 128],
                                    rec_rhs(si, d, g0, gw, k, jb, s),
                                    start=False,
                                    stop=(k == 1),
                                    skip_group_check=True,
                                )
                    if next_ops:
                        L = len(next_ops)
                        for op in next_ops[L * s // U : L * (s + 1) // U]:
                            op()
                    sp = s % 2
                    for si, (d, g0, gw) in enumerate(streams):
                        nc.scalar.activation(
                            sg[(si, sp)][:],
                            pt[(si, parity)][:, :, ds(s * gw, gw)],
                            AF.Sigmoid,
                        )
                    for si in range(ns):
                        nc.gpsimd.tensor_mul(
                            ta[(si, sp)][:], sg[(si, sp)][:, 2:4, :], ct[si][:]
                        )
                    for si in range(ns):
                        nc.vector.scalar_tensor_tensor(
                            tb2[(si, sp)][:],
                            sg[(si, sp)][:, 6:8, :],
                            0.5,
                            sg[(si, sp)][:, 0:2, :],
                            ALU.subtract,
                            ALU.mult,
                        )
                    for si in range(ns):
                        nc.vector.scalar_tensor_tensor(
                            ct[si][:],
                            tb2[(si, sp)][:],
                            2.0,
                            ta[(si, sp)][:],
                            ALU.mult,
                            ALU.add,
                        )
                    for si in range(ns):
                        nc.scalar.activation(tcb[(si, sp)][:], ct[si][:], AF.Tanh)
                    for si, (d, g0, gw) in enumerate(streams):
                        o_sl = sg[(si, sp)][:, 4:6, :]
                        if l == 0:
                            if d == 0:
                                dst = hbufF[:, :, ds(jb * UB + s * B + g0, gw)]
                            else:
                                dst = hbufR[
                                    :, :, ds((T - 1) * B - jb * UB - s * B + g0, gw)
                                ]
                            nc.gpsimd.tensor_mul(dst, o_sl, tcb[(si, sp)][:])
                        else:
                            slot = slots[si][sp]
                            nc.gpsimd.tensor_mul(slot[:], o_sl, tcb[(si, sp)][:])
                            if store_l1:
                                for jh in range(2):
                                    col = (
                                        ds(jb * UB + s * B + g0, gw)
                                        if d == 0
                                        else ds(
                                            (T - 1) * B - jb * UB - s * B + g0, gw
                                        )
                                    )
                                    nc.sync.dma_start(
                                        xin2[2 * d + jh, :, col], slot[:, jh, :]
                                    )

            # ---- peel block 0 (proj 0+1, rec 0), loop blocks 1..NBLK-2, tail ----
            mv_dma(0, 0)
            for op in proj_ops(0, 0):
                op()
            mv_dma(1, 1)
            emit_block(0, 0, proj_ops(1, 1))
            with tc.For_i(
                1, NBLK - 1, 2, hint_engines=(PE,), staggered_reset=True
            ) as jb:
                mv_dma(jb + 1, 0)
                emit_block(jb, 1, proj_ops(jb + 1, 0))
                mv_dma(jb + 2, 1)
                emit_block(jb + 1, 0, proj_ops(jb + 2, 1))
            emit_block(NBLK - 1, 1, None)
            if final_dma is not None:
                dst, si = final_dma
                src = slots[si][(T - 1) % 2]
                nc.sync.dma_start(dst[:], src[:].rearrange("p c b -> p (c b)"))
            return wt, slots, ct, pt, sg, ta, tb2, tcb

        # ---- layer 0: fwd+rev streams, history to SBUF ----
        phase(0, [(0, 0, 32), (1, 0, 32)])
        # ---- layer 1: fwd+rev streams, outputs to DRAM xin2 ----
        phase(1, [(0, 0, 32), (1, 0, 32)], store_l1=True)
        # ---- layer 2 fwd: two batch-half streams ----
        wt2, slots2, ct2, pt2, sg2, ta2, tb22, tcb2 = phase(
            2, [(0, 0, 16), (0, 16, 16)]
        )
        # assemble h2f from the two half-batch slots (parity of T-1)
        sl_p = (T - 1) % 2
        for si, g0 in ((0, 0), (1, 16)):
            for jh in range(2):
                nc.sync.dma_start(
                    out_h2f[:, ds(jh * B + g0, 16)], slots2[si][sl_p][:, jh, :]
                )

        # ---- layer 2 rev: only its first step (t = T-1) feeds the output ----
        kch = 4
        ptr = pspool.tile([128, 8, UB], f32, tag="pt00", name="ptr")
        sgr = spool.tile([128, 8, 32], f32, tag="sg00", name="sgr")
        tar = spool.tile([128, 2, 32], f32, tag="ta00", name="tar")
        tbr = spool.tile([128, 2, 32], f32, tag="tb00", name="tbr")
        tcr = spool.tile([128, 2, 32], f32, tag="tc00", name="tcr")
        cr = stpool.tile([128, 2, 32], f32, tag="c0", name="cr")
        hr = stpool.tile([128, 2, 32], bf16, tag="sl00", name="hr")
        nc.vector.memset(cr[:], 0.0)
        mvr = {}
        for k in range(kch):
            mvr[k] = mvpool.tile([128, UB], bf16, tag=f"mv1{k}0", name=f"mvr{k}")
            nc.sync.dma_start(mvr[k][:], xin2[k, :, ds((NBLK - 1) * UB, UB)])
        _, _, b_t = wt2[1]
        wih_t = wt2[1][0]
        whh_t = wt2[1][1]
        bv = b_t[:].rearrange("p (m c) -> p m c", c=UB)
        for half in range(2):
            nc.tensor.matmul(
                ptr[:, 4 * half : 4 * half + 4, :],
                ident[:],
                bv[:, 4 * half : 4 * half + 4, :],
                start=True,
                stop=False,
                skip_group_check=True,
            )
        for k in range(kch):
            rhs = mvr[k][:].rearrange("p (t b) -> p t b", b=32)[:, ::-1, :]
            for m in range(8):
                nc.tensor.matmul(
                    ptr[:, m, :],
                    wih_t[:, (k * 8 + m) * 128 : (k * 8 + m + 1) * 128],
                    rhs,
                    start=False,
                    stop=False,
                    skip_group_check=True,
                )
        for m in range(8):
            for k in range(2):
                nc.tensor.matmul(
                    ptr[:, m, 0:32],
                    whh_t[:, (k * 8 + m) * 128 : (k * 8 + m + 1) * 128],
                    zeros[:, k, :],
                    start=False,
                    stop=(k == 1),
                    skip_group_check=True,
                )
        nc.scalar.activation(sgr[:], ptr[:, :, 0:32], AF.Sigmoid)
        nc.gpsimd.tensor_mul(tar[:], sgr[:, 2:4, :], cr[:])
        nc.vector.scalar_tensor_tensor(
            tbr[:], sgr[:, 6:8, :], 0.5, sgr[:, 0:2, :], ALU.subtract, ALU.mult
        )
        nc.vector.scalar_tensor_tensor(
            cr[:], tbr[:], 2.0, tar[:], ALU.mult, ALU.add
        )
        nc.scalar.activation(tcr[:], cr[:], AF.Tanh)
        nc.gpsimd.tensor_mul(hr[:], sgr[:, 4:6, :], tcr[:])
        nc.sync.dma_start(out_h2r[:], hr[:].rearrange("p c b -> p (c b)"))

        if debug_taps:
            nc.sync.dma_start(dbg["F"][:], hbufF[:])
            nc.sync.dma_start(dbg["R"][:], hbufR[:])
            nc.sync.dma_start(dbg["X2"][:], xin2[:])


def _make_in_maps(inputs, T=TFULL, B=32, ncores=NCORES):
    x = np.ascontiguousarray(np.asarray(inputs["x"], np.float32))
    shared = {}
    for l in range(3):
        for d, dn in enumerate("fr"):
            shared[f"wih{l}{dn}"] = _prep_wih(inputs[f"wih{l}{dn}"])
            shared[f"whh{l}{dn}"] = _prep_wih(inputs[f"whh{l}{dn}"])
            shared[f"b{l}{dn}"] = _prep_b(inputs[f"b{l}{dn}"])
    shared["ident"] = np.eye(128, dtype=np.float32).astype(ml_dtypes.bfloat16)
    in_maps = []
    for ci in range(ncores):
        xs = x[ci * B : (ci + 1) * B, :T]  # [B, T, 16]
        xt = np.ascontiguousarray(xs.transpose(2, 1, 0).reshape(I0, T * B))
        m = dict(shared)
        m["xT0"] = xt.astype(ml_dtypes.bfloat16)
        in_maps.append(m)
    return in_maps


def _assemble(results, inputs, B=32):
    fcw = np.asarray(inputs["fcw"], np.float32)[0]
    fcb = float(np.asarray(inputs["fcb"], np.float32)[0])
    out = np.empty(len(results) * B, np.float32)
    for ci, r in enumerate(results):
        h2fr = np.asarray(r["h2f"], np.float32)
        h2rr = np.asarray(r["h2r"], np.float32)
        h2f = np.concatenate([h2fr[:, :B], h2fr[:, B:]], axis=0)
        h2r = np.concatenate([h2rr[:, :B], h2rr[:, B:]], axis=0)
        out[ci * B : (ci + 1) * B] = fcw[:256] @ h2f + fcw[256:] @ h2r + fcb
    return out


def kernel(**inputs):
    nc = bacc.Bacc(
        "TRN2", target_bir_lowering=False, debug=False, num_devices=NCORES
    )
    build(nc)
    nc.compile()
    in_maps = _make_in_maps(inputs)
    trace = os.environ.get("KERNEL_TRACE", "0") == "1"
    res = bass_utils.run_bass_kernel_spmd(
        nc, in_maps, core_ids=list(range(NCORES)), trace=trace
    )
    if trace and res.exec_time_ns is not None:
        print(f"HW exec time: {res.exec_time_ns} ns")
        if res.instructions_and_trace is not None:
            print(f"trace path: {res.instructions_and_trace[1]}")
    return _assemble(res.results, inputs)
